# revision 26
# baseline (speedup 1.0000x reference)
"""Trainium2 SPMD kernel for AdaptiveMultimodalFusion (8 NeuronCores, data parallel).

Math notes (exact simplifications of the reference):
  - Each _mha_seq1 has seq_len 1, so softmax over the single key is exactly 1.0
    and the MHA output is (x_kv @ wv + bv) @ wo + bo -- independent of x_q and
    of the q/k projections.
  - Chained MHAs therefore collapse to the LAST one in each chain:
        attended_image    = f(proj_clinical; attn[image_clinical])
        attended_text     = f(proj_clinical; attn[text_clinical])
        attended_clinical = f(proj_text;     attn[clinical_text])
  - Everything up to the first LayerNorm is affine in the features, so it folds
    (on host, in float64) into two matrices + one bias:
        z = feat_clinical @ Mc + feat_text @ Mt + zb          [B, 1024]
    with h = gelu(LN(z) * g1 + be1), fused = LN(h @ W2 + b2) * g2 + be2.
  - Uncertainty heads cannot fold (relu/softplus): per modality
        u_m = mean(softplus(relu(feat_m @ w1 + b1) @ w2 + b2))
    Each core reduces its batch shard to a partial sum; host finishes the mean.

Sharding: batch 16384 split 8 ways (2048 rows/core), all parameters replicated.
All device inputs are pre-swizzled on host into the exact SBUF tile layout
[partition, k-tile, free] so each DMA is one long contiguous run per partition
(the IO-DGE path is descriptor-count-bound, not bandwidth-bound).
Matmuls run in bf16 with fp32 PSUM accumulation; LayerNorm / activations /
reductions run in fp32. The LN rsqrt is a bit-trick + one Newton step on the
DVE (no ACT table traffic); softplus = ln(1+exp(x)) runs once at the tail so
the gelu ACT table stays resident for the whole kernel body.
"""

import numpy as np
import ml_dtypes

BF16 = ml_dtypes.bfloat16

N_CORES = 8
B = 16384
BS = B // N_CORES  # batch rows per core
D_IMG, D_TXT, D_CLI = 2048, 768, 256
FUSION = 512
NF1 = 2 * FUSION  # first fusion layer width
CH = 512  # batch columns per chunk on device
NCH = BS // CH  # chunks per core
NT = CH // 128  # batch tiles per chunk
LN_EPS = 1e-5

KI, KT, KC = D_IMG // 128, D_TXT // 128, D_CLI // 128  # 16, 6, 2
KW2 = NF1 // 128  # 8

MODS = ["image", "text", "clinical"]


def _build(flags):
    """Build the per-core Bass graph. flags: dict of triviality flags."""
    import concourse.mybir as mybir
    import concourse.tile as tile
    from concourse import bacc
    from concourse.masks import make_identity
    from contextlib import ExitStack

    BF = mybir.dt.bfloat16
    F32 = mybir.dt.float32
    AF = mybir.ActivationFunctionType
    ALU = mybir.AluOpType

    nc = bacc.Bacc("TRN2", target_bir_lowering=False, debug=False,
                   num_devices=N_CORES)

    # ---- DRAM I/O (pre-swizzled [partition, ktile, free] layouts) ----
    fiT = nc.dram_tensor("fiT", [NCH, 128, KI, CH], BF, kind="ExternalInput")
    ftT = nc.dram_tensor("ftT", [NCH, 128, KT, CH], BF, kind="ExternalInput")
    fcT = nc.dram_tensor("fcT", [NCH, 128, KC, CH], BF, kind="ExternalInput")
    Mc_d = nc.dram_tensor("Mc", [128, KC, NF1], BF, kind="ExternalInput")
    Mt_d = nc.dram_tensor("Mt", [128, KT, NF1], BF, kind="ExternalInput")
    W2_d = nc.dram_tensor("W2", [128, KW2, FUSION], BF, kind="ExternalInput")
    w1i_d = nc.dram_tensor("w1i", [128, KI, 128], BF, kind="ExternalInput")
    w1t_d = nc.dram_tensor("w1t", [128, KT, 128], BF, kind="ExternalInput")
    w1c_d = nc.dram_tensor("w1c", [128, KC, 128], BF, kind="ExternalInput")
    w2u_d = nc.dram_tensor("w2u", [128, 3], BF, kind="ExternalInput")
    b1u_d = nc.dram_tensor("b1u", [128, 3], F32, kind="ExternalInput")
    b2u_d = nc.dram_tensor("b2u", [3, 1], F32, kind="ExternalInput")
    # General-path parameters (only read when the fast-path flags are off).
    zb_d = nc.dram_tensor("zb", [1, NF1], BF, kind="ExternalInput")
    b2f_d = nc.dram_tensor("b2f", [1, FUSION], BF, kind="ExternalInput")
    g1_d = nc.dram_tensor("g1", [1, NF1], F32, kind="ExternalInput")
    be1_d = nc.dram_tensor("be1", [1, NF1], F32, kind="ExternalInput")
    g2_d = nc.dram_tensor("g2", [1, FUSION], F32, kind="ExternalInput")
    be2_d = nc.dram_tensor("be2", [1, FUSION], F32, kind="ExternalInput")

    out_f = nc.dram_tensor("out_fused", [BS, FUSION], F32, kind="ExternalOutput")
    out_u = nc.dram_tensor("out_unc", [3, 1], F32, kind="ExternalOutput")

    with tile.TileContext(nc) as tc, ExitStack() as ctx:
        wpool = ctx.enter_context(tc.tile_pool(name="weights", bufs=1))
        fpool = ctx.enter_context(tc.tile_pool(name="feats", bufs=2))
        tpool = ctx.enter_context(tc.tile_pool(name="temps", bufs=3))
        spool = ctx.enter_context(tc.tile_pool(name="stats", bufs=6))
        ps_mm = ctx.enter_context(tc.tile_pool(name="ps_mm", bufs=2, space="PSUM"))
        ps_tr = ctx.enter_context(tc.tile_pool(name="ps_tr", bufs=2, space="PSUM"))

        # ---- weight tiles; z-path loads on the sync queue, unc-path loads
        # on the gpsimd queue so the first z matmuls unblock early ----
        fc0_sb = fpool.tile([128, KC, CH], BF, tag="fc")
        nc.sync.dma_start(out=fc0_sb, in_=fcT[0])
        ft0_sb = fpool.tile([128, KT, CH], BF, tag="ft")
        nc.sync.dma_start(out=ft0_sb, in_=ftT[0])
        Mc_sb = wpool.tile([128, KC, NF1], BF, tag="Mc")
        nc.sync.dma_start(out=Mc_sb, in_=Mc_d[:, :, :])
        Mt_sb = wpool.tile([128, KT, NF1], BF, tag="Mt")
        for j0 in range(0, KT, 2):
            nc.sync.dma_start(out=Mt_sb[:, j0:j0 + 2, :],
                              in_=Mt_d[:, j0:j0 + 2, :])
        fi0_sb = fpool.tile([128, KI, CH], BF, tag="fi")
        for j0 in range(0, KI, 8):
            nc.gpsimd.dma_start(out=fi0_sb[:, j0:j0 + 8, :],
                                in_=fiT[0][:, j0:j0 + 8, :])
        w1i_sb = wpool.tile([128, KI, 128], BF, tag="w1i")
        nc.gpsimd.dma_start(out=w1i_sb, in_=w1i_d[:, :, :])
        w1t_sb = wpool.tile([128, KT, 128], BF, tag="w1t")
        nc.gpsimd.dma_start(out=w1t_sb, in_=w1t_d[:, :, :])
        w1c_sb = wpool.tile([128, KC, 128], BF, tag="w1c")
        nc.gpsimd.dma_start(out=w1c_sb, in_=w1c_d[:, :, :])
        W2_sb = wpool.tile([128, KW2, FUSION], BF, tag="W2")
        nc.gpsimd.dma_start(out=W2_sb, in_=W2_d[:, :, :])
        w2u_sb = wpool.tile([128, 3], BF, tag="w2u")
        nc.gpsimd.dma_start(out=w2u_sb, in_=w2u_d[:, :])
        b1u_sb = wpool.tile([128, 3], F32, tag="b1u")
        nc.gpsimd.dma_start(out=b1u_sb, in_=b1u_d[:, :])
        b2u_sb = wpool.tile([3, 1], F32, tag="b2u")
        nc.gpsimd.dma_start(out=b2u_sb, in_=b2u_d[:, :])

        ident = wpool.tile([128, 128], BF, tag="ident")
        make_identity(nc, ident)
        magic_sb = wpool.tile([128, 1], mybir.dt.int32, tag="magic")
        nc.vector.memset(magic_sb, 0x5F375A86)
        acc_sb = wpool.tile([3, 1], F32, tag="acc")
        # softplus staging: partition = modality, free = (chunk, batch col)
        sp_all = wpool.tile([3, NCH * CH], F32, tag="sp_all")

        need_ones = (not flags["zb_triv"]) or (not flags["b2f_triv"])
        if need_ones:
            ones_sb = wpool.tile([1, 128], BF, tag="ones")
            nc.vector.memset(ones_sb, 1.0)
        if not flags["zb_triv"]:
            zb_sb = wpool.tile([1, NF1], BF, tag="zb")
            nc.sync.dma_start(out=zb_sb, in_=zb_d[:, :])
        if not flags["b2f_triv"]:
            b2f_sb = wpool.tile([1, FUSION], BF, tag="b2f")
            nc.sync.dma_start(out=b2f_sb, in_=b2f_d[:, :])
        if not flags["g1_triv"]:
            g1_sb = wpool.tile([128, NF1], F32, tag="g1")
            nc.sync.dma_start(out=g1_sb, in_=g1_d.to_broadcast([128, NF1]))
            be1_sb = wpool.tile([128, NF1], F32, tag="be1")
            nc.sync.dma_start(out=be1_sb, in_=be1_d.to_broadcast([128, NF1]))
        if not flags["g2_triv"]:
            g2_sb = wpool.tile([128, FUSION], F32, tag="g2")
            nc.sync.dma_start(out=g2_sb, in_=g2_d.to_broadcast([128, FUSION]))
            be2_sb = wpool.tile([128, FUSION], F32, tag="be2")
            nc.sync.dma_start(out=be2_sb, in_=be2_d.to_broadcast([128, FUSION]))

        def rsqrt_dve(var_col):
            """rstd = 1/sqrt(var + eps): bit-trick seed + one Newton-Raphson
            step (~2e-3 rel err max) on the DVE -- no ACT table traffic."""
            v = spool.tile([128, 1], mybir.dt.float32, tag="rs_v")
            nc.vector.tensor_scalar(out=v, in0=var_col, scalar1=LN_EPS,
                                    scalar2=None, op0=ALU.add)
            yi = spool.tile([128, 1], mybir.dt.int32, tag="rs_i")
            nc.vector.tensor_scalar(out=yi, in0=v.bitcast(mybir.dt.int32),
                                    scalar1=1, scalar2=None,
                                    op0=ALU.logical_shift_right)
            nc.vector.tensor_tensor(out=yi, in0=magic_sb, in1=yi,
                                    op=ALU.subtract)
            y0 = yi.bitcast(mybir.dt.float32)
            t = spool.tile([128, 1], mybir.dt.float32, tag="rs_t")
            nc.vector.tensor_tensor(out=t, in0=v, in1=y0, op=ALU.mult)
            nc.vector.tensor_tensor(out=t, in0=t, in1=y0, op=ALU.mult)
            nc.vector.tensor_scalar(out=t, in0=t, scalar1=-0.5, scalar2=1.5,
                                    op0=ALU.mult, op1=ALU.add)
            rstd = spool.tile([128, 1], mybir.dt.float32, tag="rs_y")
            nc.vector.tensor_tensor(out=rstd, in0=y0, in1=t, op=ALU.mult)
            return rstd

        def layernorm_apply(zt, nsub, t_out, g_triv, g_sb_, be_sb_):
            """zt: PSUM tile [128, nsub, 512] (or [128, width] when nsub==1).
            Writes the normalized (optionally affine-transformed) result to
            t_out (SBUF) in one fused DVE pass."""
            stat = spool.tile([128, nsub, 6], mybir.dt.float32, tag="stat")
            for si in range(nsub):
                nc.vector.bn_stats(stat[:, si, :],
                                   zt[:, si, :] if nsub > 1 else zt)
            mv = spool.tile([128, 2], mybir.dt.float32, tag="mv")
            nc.vector.bn_aggr(mv, stat)
            rstd = rsqrt_dve(mv[:, 1:2])
            if g_triv:
                nc.vector.tensor_scalar(
                    out=t_out, in0=zt, scalar1=mv[:, 0:1], scalar2=rstd,
                    op0=ALU.subtract, op1=ALU.mult)
            else:
                width = t_out.shape[-1]
                tmp = spool.tile([128, width], mybir.dt.float32, tag="lntmp")
                nc.vector.tensor_scalar(
                    out=tmp, in0=zt, scalar1=mv[:, 0:1], scalar2=rstd,
                    op0=ALU.subtract, op1=ALU.mult)
                nc.vector.tensor_mul(tmp, tmp, g_sb_)
                nc.vector.tensor_add(t_out, tmp, be_sb_)

        def emit_z(fc_sb, ft_sb, i):
            """z matmuls for one 128-row batch tile into a 2-bank PSUM tile;
            bank 0 finishes first so LN stats can start early."""
            isl = slice(i * 128, (i + 1) * 128)
            zt = ps_mm.tile([128, 2, FUSION], mybir.dt.float32, tag="z")
            ksrc = ([(fc_sb, k, Mc_sb, k) for k in range(KC)]
                    + [(ft_sb, k, Mt_sb, k) for k in range(KT)])
            nz = len(ksrc) + (0 if flags["zb_triv"] else 1)
            for half in (0, 1):
                zp = zt[:, half, :]
                for ki, (fsb, kk, msb, mk) in enumerate(ksrc):
                    nc.tensor.matmul(zp, fsb[:, kk, isl],
                                     msb[:, mk, half * FUSION:(half + 1) * FUSION],
                                     start=(ki == 0), stop=(ki == nz - 1))
                if not flags["zb_triv"]:
                    nc.tensor.matmul(zp, ones_sb,
                                     zb_sb[:, half * FUSION:(half + 1) * FUSION],
                                     start=False, stop=True)
            return zt

        def emit_ln1(zt):
            t_sb = tpool.tile([128, NF1], BF, tag="t")
            layernorm_apply(zt, 2, t_sb, flags["g1_triv"],
                            None if flags["g1_triv"] else g1_sb,
                            None if flags["g1_triv"] else be1_sb)
            return t_sb

        def emit_tr(t_sb):
            tr_ps = ps_tr.tile([128, KW2, 128], BF, tag="aux")
            for j in range(KW2):
                nc.tensor.transpose(tr_ps[:, j, :],
                                    t_sb[:, j * 128:(j + 1) * 128], ident)
            return tr_ps

        def emit_gelu(tr_ps):
            hT = tpool.tile([128, KW2, 128], BF, tag="hT")
            nc.scalar.activation(hT, tr_ps, func=AF.Gelu)
            return hT

        def emit_y_out(hT, row0):
            yp = ps_mm.tile([128, FUSION], mybir.dt.float32, tag="y", bufs=1)
            ny = KW2 + (0 if flags["b2f_triv"] else 1)
            for j in range(KW2):
                nc.tensor.matmul(yp, hT[:, j, :], W2_sb[:, j, :],
                                 start=(j == 0), stop=(j == ny - 1))
            if not flags["b2f_triv"]:
                nc.tensor.matmul(yp, ones_sb, b2f_sb, start=False, stop=True)
            o_sb = tpool.tile([128, FUSION], mybir.dt.float32, tag="o")
            layernorm_apply(yp, 1, o_sb, flags["g2_triv"],
                            None if flags["g2_triv"] else g2_sb,
                            None if flags["g2_triv"] else be2_sb)
            nc.sync.dma_start(out=out_f[row0:row0 + 128, :], in_=o_sb)

        def unc_filler(c, fi_sb, ft_sb, fc_sb):
            """Generator emitting one chunk's uncertainty-head work in small
            pieces; drained between fusion stages as PE filler."""
            for m, (fsb, nk, w1sb) in enumerate(
                    [(fi_sb, KI, w1i_sb), (ft_sb, KT, w1t_sb),
                     (fc_sb, KC, w1c_sb)]):
                hm_ps = ps_tr.tile([128, CH], mybir.dt.float32, tag="aux")
                for k in range(nk):
                    nc.tensor.matmul(hm_ps, w1sb[:, k, :], fsb[:, k, :],
                                     start=(k == 0), stop=(k == nk - 1))
                    yield
                hm_sb = tpool.tile([128, CH], BF, tag="hm")
                nc.scalar.activation(hm_sb, hm_ps, func=AF.Relu,
                                     bias=b1u_sb[:, m:m + 1], scale=1.0)
                sp_ps = ps_tr.tile([1, CH], mybir.dt.float32, tag="s3", bufs=1)
                nc.tensor.matmul(sp_ps, w2u_sb[:, m:m + 1], hm_sb,
                                 start=True, stop=True)
                yield
                sp_st = spool.tile([1, CH], mybir.dt.float32, tag="sp_st")
                nc.scalar.activation(sp_st, sp_ps, func=AF.Copy)
                nc.gpsimd.dma_start(out=sp_all[m:m + 1, c * CH:(c + 1) * CH],
                                    in_=sp_st)
                yield

        def emit_softplus_tail():
            """softplus(x) = ln(1 + exp(x)); exp and ln live in one ACT table
            set, and accum_out reduces each modality over the batch axis."""
            e_all = wpool.tile([3, NCH * CH], mybir.dt.float32, tag="e_all")
            nc.scalar.activation(e_all, sp_all, func=AF.Exp,
                                 bias=b2u_sb[:, 0:1], scale=1.0)
            spv = wpool.tile([3, NCH * CH], mybir.dt.float32, tag="spv")
            nc.scalar.activation(spv, e_all, func=AF.Ln, bias=1.0, scale=1.0,
                                 accum_out=acc_sb[:, 0:1])
            nc.gpsimd.dma_start(out=out_u[:, :], in_=acc_sb)

        feats = {0: (fc0_sb, ft0_sb, fi0_sb)}
        filler = None
        done = [True]

        def drain(n):
            for _ in range(n):
                if next(filler, "END") == "END":
                    done[0] = True
                    return

        NTOT = NCH * NT
        z = None
        for t in range(NTOT):
            c, i = divmod(t, NT)
            if i == 0:
                if c + 1 < NCH:
                    nf = fpool.tile([128, KC, CH], BF, tag="fc")
                    nc.sync.dma_start(out=nf, in_=fcT[c + 1])
                    nft = fpool.tile([128, KT, CH], BF, tag="ft")
                    nc.sync.dma_start(out=nft, in_=ftT[c + 1])
                    nfi = fpool.tile([128, KI, CH], BF, tag="fi")
                    nc.gpsimd.dma_start(out=nfi, in_=fiT[c + 1])
                    feats[c + 1] = (nf, nft, nfi)
                filler = unc_filler(c, feats[c][2], feats[c][1], feats[c][0])
                done = [False]
            if z is None:
                z = emit_z(feats[0][0], feats[0][1], 0)
            if t == NTOT - 1:
                # finish the uncertainty work, then softplus while the last
                # tile's fusion stages still occupy PE/DVE
                while not done[0]:
                    drain(1)
                emit_softplus_tail()
            t_sb = emit_ln1(z)
            if t + 1 < NTOT:
                nxt_c, nxt_i = divmod(t + 1, NT)
                z = emit_z(feats[nxt_c][0], feats[nxt_c][1], nxt_i)
            else:
                drain(10)
            tr_ps = emit_tr(t_sb)
            drain(3)
            hT = emit_gelu(tr_ps)
            emit_y_out(hT, t * 128)
            drain(2)
            if i == NT - 1:
                # finish this chunk's uncertainty work so its feature tiles
                # release before the next prefetch needs the pool slots
                while not done[0]:
                    drain(1)
                feats.pop(c, None)

    nc.finalize()
    return nc


def _fold_params(params):
    """Fold all pre-LN1 linear algebra into Mc/Mt/zb (float64 on host)."""
    f64 = lambda a: np.asarray(a, dtype=np.float64)
    Wp = {m: f64(params["proj"][m]["w"]) for m in MODS}
    bp = {m: f64(params["proj"][m]["b"]) for m in MODS}

    def fold_pair(src, key):
        a = params["attn"][key]
        wv, bv = f64(a["wv"]), f64(a["bv"])
        wo, bo = f64(a["wo"]), f64(a["bo"])
        G = Wp[src] @ wv @ wo
        g = (bp[src] @ wv + bv) @ wo + bo
        return G, g

    G_img, g_img = fold_pair("clinical", "image_clinical")
    G_txt, g_txt = fold_pair("clinical", "text_clinical")
    G_cli, g_cli = fold_pair("text", "clinical_text")

    fus = params["fus"]
    W1 = f64(fus["w1"])
    b1 = f64(fus["b1"])
    Mc = G_img @ W1[0:FUSION] + G_txt @ W1[FUSION:2 * FUSION]
    Mt = G_cli @ W1[2 * FUSION:3 * FUSION]
    zb = (g_img @ W1[0:FUSION] + g_txt @ W1[FUSION:2 * FUSION]
          + g_cli @ W1[2 * FUSION:3 * FUSION] + b1)
    return Mc, Mt, zb


def _swz_w(a, nk):
    """[D, N] -> bf16 [128, nk, N] (partition-major, contiguous)."""
    a = np.asarray(a)
    return np.ascontiguousarray(
        a.reshape(nk, 128, a.shape[1]).transpose(1, 0, 2)).astype(BF16)


def _swz_feat(x):
    """Per-core feature slice [BS, D] f32 -> bf16 [NCH, 128, D//128, CH]."""
    xT = np.ascontiguousarray(x.T)  # [D, BS]
    nk = xT.shape[0] // 128
    sw = xT.reshape(nk, 128, NCH, CH).transpose(2, 1, 0, 3)
    return np.ascontiguousarray(sw).astype(BF16)


_CACHE = {}


def kernel(feat_image, feat_text, feat_clinical, params):
    from concourse.bass_utils import run_bass_kernel_spmd

    feat_image = np.asarray(feat_image, dtype=np.float32)
    feat_text = np.asarray(feat_text, dtype=np.float32)
    feat_clinical = np.asarray(feat_clinical, dtype=np.float32)

    Mc, Mt, zb = _fold_params(params)
    fus = params["fus"]
    f32 = lambda a: np.asarray(a, dtype=np.float32)
    b2f = f32(fus["b2"])
    g1, be1 = f32(fus["g1"]), f32(fus["be1"])
    g2, be2 = f32(fus["g2"]), f32(fus["be2"])
    unc = params["unc"]
    b1u = np.ascontiguousarray(
        np.stack([f32(unc[m]["b1"]) for m in MODS], axis=1))  # [128, 3]
    w2u = np.stack([f32(unc[m]["w2"]).reshape(128) for m in MODS], axis=1)
    b2u = np.asarray([f32(unc[m]["b2"]).reshape(()) for m in MODS],
                     dtype=np.float32).reshape(3, 1)

    flags = {
        "zb_triv": bool(np.all(zb == 0.0)),
        "b2f_triv": bool(np.all(b2f == 0.0)),
        "g1_triv": bool(np.all(g1 == 1.0) and np.all(be1 == 0.0)),
        "g2_triv": bool(np.all(g2 == 1.0) and np.all(be2 == 0.0)),
    }

    key = tuple(sorted(flags.items()))
    if key not in _CACHE:
        _CACHE[key] = _build(flags)
    nc = _CACHE[key]

    bf = lambda a: np.ascontiguousarray(a).astype(BF16)
    shared = {
        "Mc": _swz_w(Mc, KC), "Mt": _swz_w(Mt, KT),
        "W2": _swz_w(f32(fus["w2"]), KW2),
        "w1i": _swz_w(f32(unc["image"]["w1"]), KI),
        "w1t": _swz_w(f32(unc["text"]["w1"]), KT),
        "w1c": _swz_w(f32(unc["clinical"]["w1"]), KC),
        "w2u": bf(w2u), "b1u": b1u, "b2u": b2u,
        "zb": bf(zb.reshape(1, -1)), "b2f": bf(b2f.reshape(1, -1)),
        "g1": g1.reshape(1, -1), "be1": be1.reshape(1, -1),
        "g2": g2.reshape(1, -1), "be2": be2.reshape(1, -1),
    }
    in_maps = []
    for c in range(N_CORES):
        sl = slice(c * BS, (c + 1) * BS)
        in_maps.append({
            "fiT": _swz_feat(feat_image[sl]),
            "ftT": _swz_feat(feat_text[sl]),
            "fcT": _swz_feat(feat_clinical[sl]),
            **shared,
        })

    res = run_bass_kernel_spmd(nc, in_maps, core_ids=list(range(N_CORES)))

    fused = np.concatenate(
        [np.asarray(res.results[c]["out_fused"]) for c in range(N_CORES)], axis=0)
    usum = np.zeros(3, dtype=np.float64)
    for c in range(N_CORES):
        usum += np.asarray(res.results[c]["out_unc"], dtype=np.float64).reshape(3)
    uncertainties = (usum / B).astype(np.float32)
    return fused.astype(np.float32), uncertainties


# revision 35
# speedup vs baseline: 1.5855x; 1.5855x over previous
"""Trainium2 SPMD kernel for AdaptiveMultimodalFusion (8 NeuronCores, data parallel).

Math notes (exact simplifications of the reference):
  - Each _mha_seq1 has seq_len 1, so softmax over the single key is exactly 1.0
    and the MHA output is (x_kv @ wv + bv) @ wo + bo -- independent of x_q and
    of the q/k projections.
  - Chained MHAs therefore collapse to the LAST one in each chain:
        attended_image    = f(proj_clinical; attn[image_clinical])
        attended_text     = f(proj_clinical; attn[text_clinical])
        attended_clinical = f(proj_text;     attn[clinical_text])
  - Everything up to the first LayerNorm is affine in the features, so it folds
    (on host, in float64) into two matrices + one bias:
        z = feat_clinical @ Mc + feat_text @ Mt + zb          [B, 1024]
    with h = gelu(LN(z) * g1 + be1), fused = LN(h @ W2 + b2) * g2 + be2.
  - Uncertainty heads cannot fold (relu/softplus): per modality
        u_m = mean(softplus(relu(feat_m @ w1 + b1) @ w2 + b2))
    Each core reduces its batch shard to a partial sum; host finishes the mean.

Sharding: batch 16384 split 8 ways (2048 rows/core), all parameters replicated.
All device inputs are pre-swizzled on host into the exact SBUF tile layout
[partition, k-tile, free] so each DMA is one long contiguous run per partition
(the IO-DGE path is descriptor-count-bound, not bandwidth-bound).
Matmuls run in bf16 with fp32 PSUM accumulation; LayerNorm / activations /
reductions run in fp32. The LN rsqrt is a bit-trick + one Newton step on the
DVE (no ACT table traffic); softplus = ln(1+exp(x)) runs once at the tail so
the gelu ACT table stays resident for the whole kernel body.
"""

import numpy as np
import ml_dtypes

BF16 = ml_dtypes.bfloat16

N_CORES = 8
B = 16384
BS = B // N_CORES  # batch rows per core
D_IMG, D_TXT, D_CLI = 2048, 768, 256
FUSION = 512
NF1 = 2 * FUSION  # first fusion layer width
CH = 512  # batch columns per chunk on device
NCH = BS // CH  # chunks per core
NT = CH // 128  # batch tiles per chunk
LN_EPS = 1e-5

KI, KT, KC = D_IMG // 128, D_TXT // 128, D_CLI // 128  # 16, 6, 2
KW2 = NF1 // 128  # 8

MODS = ["image", "text", "clinical"]


def _build(flags):
    """Build the per-core Bass graph. flags: dict of triviality flags."""
    import concourse.mybir as mybir
    import concourse.tile as tile
    from concourse import bacc
    from concourse.masks import make_identity
    from contextlib import ExitStack

    BF = mybir.dt.bfloat16
    F32 = mybir.dt.float32
    AF = mybir.ActivationFunctionType
    ALU = mybir.AluOpType

    nc = bacc.Bacc("TRN2", target_bir_lowering=False, debug=False,
                   num_devices=N_CORES)

    # ---- DRAM I/O (pre-swizzled [partition, ktile, free] layouts) ----
    fiT = nc.dram_tensor("fiT", [NCH, 128, KI, CH], BF, kind="ExternalInput")
    ftT = nc.dram_tensor("ftT", [NCH, 128, KT, CH], BF, kind="ExternalInput")
    fcT = nc.dram_tensor("fcT", [NCH, 128, KC, CH], BF, kind="ExternalInput")
    Mc_d = nc.dram_tensor("Mc", [128, KC, NF1], BF, kind="ExternalInput")
    Mt_d = nc.dram_tensor("Mt", [128, KT, NF1], BF, kind="ExternalInput")
    W2_d = nc.dram_tensor("W2", [128, KW2, FUSION], BF, kind="ExternalInput")
    w1i_d = nc.dram_tensor("w1i", [128, KI, 128], BF, kind="ExternalInput")
    w1t_d = nc.dram_tensor("w1t", [128, KT, 128], BF, kind="ExternalInput")
    w1c_d = nc.dram_tensor("w1c", [128, KC, 128], BF, kind="ExternalInput")
    w2u_d = nc.dram_tensor("w2u", [128, 3], BF, kind="ExternalInput")
    b1u_d = nc.dram_tensor("b1u", [128, 3], F32, kind="ExternalInput")
    b2u_d = nc.dram_tensor("b2u", [3, 1], F32, kind="ExternalInput")
    # General-path parameters (only read when the fast-path flags are off).
    zb_d = nc.dram_tensor("zb", [1, NF1], BF, kind="ExternalInput")
    b2f_d = nc.dram_tensor("b2f", [1, FUSION], BF, kind="ExternalInput")
    g1_d = nc.dram_tensor("g1", [1, NF1], F32, kind="ExternalInput")
    be1_d = nc.dram_tensor("be1", [1, NF1], F32, kind="ExternalInput")
    g2_d = nc.dram_tensor("g2", [1, FUSION], F32, kind="ExternalInput")
    be2_d = nc.dram_tensor("be2", [1, FUSION], F32, kind="ExternalInput")

    out_f = nc.dram_tensor("out_fused", [BS, FUSION], F32, kind="ExternalOutput")
    out_u = nc.dram_tensor("out_unc", [3, 1], F32, kind="ExternalOutput")

    with tile.TileContext(nc) as tc, ExitStack() as ctx:
        wpool = ctx.enter_context(tc.tile_pool(name="weights", bufs=1))
        fpool = ctx.enter_context(tc.tile_pool(name="feats", bufs=2))
        tpool = ctx.enter_context(tc.tile_pool(name="temps", bufs=3))
        spool = ctx.enter_context(tc.tile_pool(name="stats", bufs=6))
        ps_mm = ctx.enter_context(tc.tile_pool(name="ps_mm", bufs=2, space="PSUM"))
        ps_tr = ctx.enter_context(tc.tile_pool(name="ps_tr", bufs=2, space="PSUM"))

        # ---- weight tiles; z-path loads on the sync queue, unc-path loads
        # on the gpsimd queue so the first z matmuls unblock early ----
        fc0_sb = fpool.tile([128, KC, CH], BF, tag="fc")
        nc.sync.dma_start(out=fc0_sb, in_=fcT[0])
        Mc_sb = wpool.tile([128, KC, NF1], BF, tag="Mc")
        nc.sync.dma_start(out=Mc_sb, in_=Mc_d[:, :, :])
        ft0_sb = fpool.tile([128, KT, CH], BF, tag="ft")
        nc.sync.dma_start(out=ft0_sb, in_=ftT[0])
        Mt_sb = wpool.tile([128, KT, NF1], BF, tag="Mt")
        nc.sync.dma_start(out=Mt_sb, in_=Mt_d[:, :, :])
        fi0_sb = fpool.tile([128, KI, CH], BF, tag="fi")
        nc.gpsimd.dma_start(out=fi0_sb, in_=fiT[0])
        w1i_sb = wpool.tile([128, KI, 128], BF, tag="w1i")
        nc.gpsimd.dma_start(out=w1i_sb, in_=w1i_d[:, :, :])
        w1t_sb = wpool.tile([128, KT, 128], BF, tag="w1t")
        nc.gpsimd.dma_start(out=w1t_sb, in_=w1t_d[:, :, :])
        w1c_sb = wpool.tile([128, KC, 128], BF, tag="w1c")
        nc.gpsimd.dma_start(out=w1c_sb, in_=w1c_d[:, :, :])
        W2_sb = wpool.tile([128, KW2, FUSION], BF, tag="W2")
        nc.gpsimd.dma_start(out=W2_sb, in_=W2_d[:, :, :])
        w2u_sb = wpool.tile([128, 3], BF, tag="w2u")
        nc.gpsimd.dma_start(out=w2u_sb, in_=w2u_d[:, :])
        b1u_sb = wpool.tile([128, 3], F32, tag="b1u")
        nc.gpsimd.dma_start(out=b1u_sb, in_=b1u_d[:, :])
        b2u_sb = wpool.tile([3, 1], F32, tag="b2u")
        nc.gpsimd.dma_start(out=b2u_sb, in_=b2u_d[:, :])

        ident = wpool.tile([128, 128], BF, tag="ident")
        make_identity(nc, ident)
        magic_sb = wpool.tile([128, 1], mybir.dt.int32, tag="magic")
        nc.vector.memset(magic_sb, 0x5F375A86)
        acc_sb = wpool.tile([3, 1], F32, tag="acc")
        # softplus staging: partition = modality, free = (chunk, batch col)
        sp_all = wpool.tile([3, NCH * CH], F32, tag="sp_all")

        need_ones = (not flags["zb_triv"]) or (not flags["b2f_triv"])
        if need_ones:
            ones_sb = wpool.tile([1, 128], BF, tag="ones")
            nc.vector.memset(ones_sb, 1.0)
        if not flags["zb_triv"]:
            zb_sb = wpool.tile([1, NF1], BF, tag="zb")
            nc.sync.dma_start(out=zb_sb, in_=zb_d[:, :])
        if not flags["b2f_triv"]:
            b2f_sb = wpool.tile([1, FUSION], BF, tag="b2f")
            nc.sync.dma_start(out=b2f_sb, in_=b2f_d[:, :])
        if not flags["g1_triv"]:
            g1_sb = wpool.tile([128, NF1], F32, tag="g1")
            nc.sync.dma_start(out=g1_sb, in_=g1_d.to_broadcast([128, NF1]))
            be1_sb = wpool.tile([128, NF1], F32, tag="be1")
            nc.sync.dma_start(out=be1_sb, in_=be1_d.to_broadcast([128, NF1]))
        if not flags["g2_triv"]:
            g2_sb = wpool.tile([128, FUSION], F32, tag="g2")
            nc.sync.dma_start(out=g2_sb, in_=g2_d.to_broadcast([128, FUSION]))
            be2_sb = wpool.tile([128, FUSION], F32, tag="be2")
            nc.sync.dma_start(out=be2_sb, in_=be2_d.to_broadcast([128, FUSION]))

        def rsqrt_dve(var_col):
            """rstd = 1/sqrt(var + eps): bit-trick seed + one Newton-Raphson
            step (~2e-3 rel err max) on the DVE -- no ACT table traffic."""
            v = spool.tile([128, 1], mybir.dt.float32, tag="rs_v")
            nc.vector.tensor_scalar(out=v, in0=var_col, scalar1=LN_EPS,
                                    scalar2=None, op0=ALU.add)
            yi = spool.tile([128, 1], mybir.dt.int32, tag="rs_i")
            nc.vector.tensor_scalar(out=yi, in0=v.bitcast(mybir.dt.int32),
                                    scalar1=1, scalar2=None,
                                    op0=ALU.logical_shift_right)
            nc.vector.tensor_tensor(out=yi, in0=magic_sb, in1=yi,
                                    op=ALU.subtract)
            y0 = yi.bitcast(mybir.dt.float32)
            t = spool.tile([128, 1], mybir.dt.float32, tag="rs_t")
            nc.vector.tensor_tensor(out=t, in0=v, in1=y0, op=ALU.mult)
            nc.vector.tensor_tensor(out=t, in0=t, in1=y0, op=ALU.mult)
            nc.vector.tensor_scalar(out=t, in0=t, scalar1=-0.5, scalar2=1.5,
                                    op0=ALU.mult, op1=ALU.add)
            rstd = spool.tile([128, 1], mybir.dt.float32, tag="rs_y")
            nc.vector.tensor_tensor(out=rstd, in0=y0, in1=t, op=ALU.mult)
            return rstd

        def layernorm_apply(zt, nsub, t_out, g_triv, g_sb_, be_sb_):
            """zt: PSUM tile [128, nsub, 512] (or [128, width] when nsub==1).
            Writes the normalized (optionally affine-transformed) result to
            t_out (SBUF) in one fused DVE pass."""
            stat = spool.tile([128, nsub, 6], mybir.dt.float32, tag="stat")
            for si in range(nsub):
                nc.vector.bn_stats(stat[:, si, :],
                                   zt[:, si, :] if nsub > 1 else zt)
            mv = spool.tile([128, 2], mybir.dt.float32, tag="mv")
            nc.vector.bn_aggr(mv, stat)
            rstd = rsqrt_dve(mv[:, 1:2])
            if g_triv:
                nc.vector.tensor_scalar(
                    out=t_out, in0=zt, scalar1=mv[:, 0:1], scalar2=rstd,
                    op0=ALU.subtract, op1=ALU.mult)
            else:
                width = t_out.shape[-1]
                tmp = spool.tile([128, width], mybir.dt.float32, tag="lntmp")
                nc.vector.tensor_scalar(
                    out=tmp, in0=zt, scalar1=mv[:, 0:1], scalar2=rstd,
                    op0=ALU.subtract, op1=ALU.mult)
                nc.vector.tensor_mul(tmp, tmp, g_sb_)
                nc.vector.tensor_add(t_out, tmp, be_sb_)

        def emit_z(fc_sb, ft_sb, i, mid_fill=None):
            """z matmuls for one 128-row batch tile into a 2-bank PSUM tile;
            bank 0 finishes first so LN stats can start early. mid_fill, if
            given, emits PE filler between the fc- and ft-contractions (used
            at startup while Mt is still in flight)."""
            isl = slice(i * 128, (i + 1) * 128)
            zt = ps_mm.tile([128, 2, FUSION], mybir.dt.float32, tag="z")
            nz = KC + KT + (0 if flags["zb_triv"] else 1)
            for half in (0, 1):
                zp = zt[:, half, :]
                nsl = slice(half * FUSION, (half + 1) * FUSION)
                for k in range(KC):
                    nc.tensor.matmul(zp, fc_sb[:, k, isl], Mc_sb[:, k, nsl],
                                     start=(k == 0), stop=False)
            if mid_fill is not None:
                mid_fill()
            for half in (0, 1):
                zp = zt[:, half, :]
                nsl = slice(half * FUSION, (half + 1) * FUSION)
                for k in range(KT):
                    nc.tensor.matmul(zp, ft_sb[:, k, isl], Mt_sb[:, k, nsl],
                                     start=False, stop=(k == KT - 1 and nz == KC + KT))
                if not flags["zb_triv"]:
                    nc.tensor.matmul(zp, ones_sb, zb_sb[:, nsl],
                                     start=False, stop=True)
            return zt

        def emit_ln1(zt):
            t_sb = tpool.tile([128, NF1], BF, tag="t")
            layernorm_apply(zt, 2, t_sb, flags["g1_triv"],
                            None if flags["g1_triv"] else g1_sb,
                            None if flags["g1_triv"] else be1_sb)
            return t_sb

        def emit_tr(t_sb):
            tr_ps = ps_tr.tile([128, KW2, 128], BF, tag="aux")
            for j in range(KW2):
                nc.tensor.transpose(tr_ps[:, j, :],
                                    t_sb[:, j * 128:(j + 1) * 128], ident)
            return tr_ps

        def emit_gelu(tr_ps):
            hT = tpool.tile([128, KW2, 128], BF, tag="hT")
            nc.scalar.activation(hT, tr_ps, func=AF.Gelu)
            return hT

        def emit_y(hT):
            yp = ps_mm.tile([128, FUSION], mybir.dt.float32, tag="y", bufs=2)
            ny = KW2 + (0 if flags["b2f_triv"] else 1)
            for j in range(KW2):
                nc.tensor.matmul(yp, hT[:, j, :], W2_sb[:, j, :],
                                 start=(j == 0), stop=(j == ny - 1))
            if not flags["b2f_triv"]:
                nc.tensor.matmul(yp, ones_sb, b2f_sb, start=False, stop=True)
            return yp

        def emit_ln2_out(yp, row0):
            o_sb = tpool.tile([128, FUSION], mybir.dt.float32, tag="o")
            layernorm_apply(yp, 1, o_sb, flags["g2_triv"],
                            None if flags["g2_triv"] else g2_sb,
                            None if flags["g2_triv"] else be2_sb)
            nc.sync.dma_start(out=out_f[row0:row0 + 128, :], in_=o_sb)

        def unc_filler(c, fi_sb, ft_sb, fc_sb):
            """Generator emitting one chunk's uncertainty-head work in small
            pieces; drained between fusion stages as PE filler."""
            for m, (fsb, nk, w1sb) in enumerate(
                    [(fi_sb, KI, w1i_sb), (ft_sb, KT, w1t_sb),
                     (fc_sb, KC, w1c_sb)]):
                hm_ps = ps_tr.tile([128, CH], mybir.dt.float32, tag="aux")
                for k in range(nk):
                    nc.tensor.matmul(hm_ps, w1sb[:, k, :], fsb[:, k, :],
                                     start=(k == 0), stop=(k == nk - 1))
                    yield
                hm_sb = tpool.tile([128, CH], BF, tag="hm")
                nc.scalar.activation(hm_sb, hm_ps, func=AF.Relu,
                                     bias=b1u_sb[:, m:m + 1], scale=1.0)
                sp_ps = ps_tr.tile([1, CH], mybir.dt.float32, tag="aux")
                nc.tensor.matmul(sp_ps, w2u_sb[:, m:m + 1], hm_sb,
                                 start=True, stop=True)
                yield
                sp_st = spool.tile([1, CH], mybir.dt.float32, tag="sp_st")
                nc.scalar.activation(sp_st, sp_ps, func=AF.Copy)
                nc.gpsimd.dma_start(out=sp_all[m:m + 1, c * CH:(c + 1) * CH],
                                    in_=sp_st)
                yield

        def emit_softplus_tail():
            """softplus(x) = ln(1 + exp(x)); exp and ln live in one ACT table
            set, and accum_out reduces each modality over the batch axis."""
            e_all = wpool.tile([3, NCH * CH], mybir.dt.float32, tag="e_all")
            nc.scalar.activation(e_all, sp_all, func=AF.Exp,
                                 bias=b2u_sb[:, 0:1], scale=1.0)
            spv = wpool.tile([3, NCH * CH], mybir.dt.float32, tag="spv")
            nc.scalar.activation(spv, e_all, func=AF.Ln, bias=1.0, scale=1.0,
                                 accum_out=acc_sb[:, 0:1])
            nc.gpsimd.dma_start(out=out_u[:, :], in_=acc_sb)

        feats = {0: (fc0_sb, ft0_sb, fi0_sb)}
        filler = None
        done = [True]

        def drain(n):
            for _ in range(n):
                if next(filler, "END") == "END":
                    done[0] = True
                    return

        NTOT = NCH * NT
        z = None
        pending = None
        for t in range(NTOT):
            c, i = divmod(t, NT)
            if i == 0:
                if c + 1 < NCH:
                    nf = fpool.tile([128, KC, CH], BF, tag="fc")
                    nc.sync.dma_start(out=nf, in_=fcT[c + 1])
                    nft = fpool.tile([128, KT, CH], BF, tag="ft")
                    nc.sync.dma_start(out=nft, in_=ftT[c + 1])
                    nfi = fpool.tile([128, KI, CH], BF, tag="fi")
                    nc.gpsimd.dma_start(out=nfi, in_=fiT[c + 1])
                    feats[c + 1] = (nf, nft, nfi)
                filler = unc_filler(c, feats[c][2], feats[c][1], feats[c][0])
                done = [False]
            if z is None:
                z = emit_z(feats[0][0], feats[0][1], 0,
                           mid_fill=lambda: drain(17))
            if t == NTOT - 1:
                # finish the uncertainty work, then softplus while the last
                # tile's fusion stages still occupy PE/DVE
                while not done[0]:
                    drain(1)
                emit_softplus_tail()
            t_sb = emit_ln1(z)
            if pending is not None:
                emit_ln2_out(*pending)
                pending = None
            if t + 1 < NTOT:
                nxt_c, nxt_i = divmod(t + 1, NT)
                z = emit_z(feats[nxt_c][0], feats[nxt_c][1], nxt_i)
            else:
                drain(10)
            tr_ps = emit_tr(t_sb)
            drain(3)
            hT = emit_gelu(tr_ps)
            if t == NTOT - 1:
                emit_ln2_out(emit_y(hT), t * 128)
            else:
                pending = (emit_y(hT), t * 128)
            drain(2)
            if i == NT - 1:
                # finish this chunk's uncertainty work so its feature tiles
                # release before the next prefetch needs the pool slots
                while not done[0]:
                    drain(1)
                feats.pop(c, None)
        if pending is not None:
            emit_ln2_out(*pending)

    nc.finalize()
    return nc


def _fold_params(params):
    """Fold all pre-LN1 linear algebra into Mc/Mt/zb (float64 on host)."""
    f64 = lambda a: np.asarray(a, dtype=np.float64)
    Wp = {m: f64(params["proj"][m]["w"]) for m in MODS}
    bp = {m: f64(params["proj"][m]["b"]) for m in MODS}

    def fold_pair(src, key):
        a = params["attn"][key]
        wv, bv = f64(a["wv"]), f64(a["bv"])
        wo, bo = f64(a["wo"]), f64(a["bo"])
        G = Wp[src] @ wv @ wo
        g = (bp[src] @ wv + bv) @ wo + bo
        return G, g

    G_img, g_img = fold_pair("clinical", "image_clinical")
    G_txt, g_txt = fold_pair("clinical", "text_clinical")
    G_cli, g_cli = fold_pair("text", "clinical_text")

    fus = params["fus"]
    W1 = f64(fus["w1"])
    b1 = f64(fus["b1"])
    Mc = G_img @ W1[0:FUSION] + G_txt @ W1[FUSION:2 * FUSION]
    Mt = G_cli @ W1[2 * FUSION:3 * FUSION]
    zb = (g_img @ W1[0:FUSION] + g_txt @ W1[FUSION:2 * FUSION]
          + g_cli @ W1[2 * FUSION:3 * FUSION] + b1)
    return Mc, Mt, zb


def _swz_w(a, nk):
    """[D, N] -> bf16 [128, nk, N] (partition-major, contiguous)."""
    a = np.asarray(a)
    return np.ascontiguousarray(
        a.reshape(nk, 128, a.shape[1]).transpose(1, 0, 2)).astype(BF16)


def _swz_feat(x):
    """Per-core feature slice [BS, D] f32 -> bf16 [NCH, 128, D//128, CH]."""
    xT = np.ascontiguousarray(x.T)  # [D, BS]
    nk = xT.shape[0] // 128
    sw = xT.reshape(nk, 128, NCH, CH).transpose(2, 1, 0, 3)
    return np.ascontiguousarray(sw).astype(BF16)


_CACHE = {}


def kernel(feat_image, feat_text, feat_clinical, params):
    from concourse.bass_utils import run_bass_kernel_spmd

    feat_image = np.asarray(feat_image, dtype=np.float32)
    feat_text = np.asarray(feat_text, dtype=np.float32)
    feat_clinical = np.asarray(feat_clinical, dtype=np.float32)

    Mc, Mt, zb = _fold_params(params)
    fus = params["fus"]
    f32 = lambda a: np.asarray(a, dtype=np.float32)
    b2f = f32(fus["b2"])
    g1, be1 = f32(fus["g1"]), f32(fus["be1"])
    g2, be2 = f32(fus["g2"]), f32(fus["be2"])
    unc = params["unc"]
    b1u = np.ascontiguousarray(
        np.stack([f32(unc[m]["b1"]) for m in MODS], axis=1))  # [128, 3]
    w2u = np.stack([f32(unc[m]["w2"]).reshape(128) for m in MODS], axis=1)
    b2u = np.asarray([f32(unc[m]["b2"]).reshape(()) for m in MODS],
                     dtype=np.float32).reshape(3, 1)

    flags = {
        "zb_triv": bool(np.all(zb == 0.0)),
        "b2f_triv": bool(np.all(b2f == 0.0)),
        "g1_triv": bool(np.all(g1 == 1.0) and np.all(be1 == 0.0)),
        "g2_triv": bool(np.all(g2 == 1.0) and np.all(be2 == 0.0)),
    }

    key = tuple(sorted(flags.items()))
    if key not in _CACHE:
        _CACHE[key] = _build(flags)
    nc = _CACHE[key]

    bf = lambda a: np.ascontiguousarray(a).astype(BF16)
    shared = {
        "Mc": _swz_w(Mc, KC), "Mt": _swz_w(Mt, KT),
        "W2": _swz_w(f32(fus["w2"]), KW2),
        "w1i": _swz_w(f32(unc["image"]["w1"]), KI),
        "w1t": _swz_w(f32(unc["text"]["w1"]), KT),
        "w1c": _swz_w(f32(unc["clinical"]["w1"]), KC),
        "w2u": bf(w2u), "b1u": b1u, "b2u": b2u,
        "zb": bf(zb.reshape(1, -1)), "b2f": bf(b2f.reshape(1, -1)),
        "g1": g1.reshape(1, -1), "be1": be1.reshape(1, -1),
        "g2": g2.reshape(1, -1), "be2": be2.reshape(1, -1),
    }
    in_maps = []
    for c in range(N_CORES):
        sl = slice(c * BS, (c + 1) * BS)
        in_maps.append({
            "fiT": _swz_feat(feat_image[sl]),
            "ftT": _swz_feat(feat_text[sl]),
            "fcT": _swz_feat(feat_clinical[sl]),
            **shared,
        })

    res = run_bass_kernel_spmd(nc, in_maps, core_ids=list(range(N_CORES)))

    fused = np.concatenate(
        [np.asarray(res.results[c]["out_fused"]) for c in range(N_CORES)], axis=0)
    usum = np.zeros(3, dtype=np.float64)
    for c in range(N_CORES):
        usum += np.asarray(res.results[c]["out_unc"], dtype=np.float64).reshape(3)
    uncertainties = (usum / B).astype(np.float32)
    return fused.astype(np.float32), uncertainties
